# revision 3
# baseline (speedup 1.0000x reference)
"""MoE MLP (top-2 of 8 experts, SwiGLU) on 8 Trainium2 NeuronCores.

Strategy (expert parallelism, per the sharding hint):
  - Host computes router logits once to decide the dispatch (sharding
    decision only), gathers each expert's tokens, and pads to a common
    capacity C.
  - Core e holds expert e's weights (bf16) and runs the 3 matmuls +
    SwiGLU over its gathered tokens, scaling by the combine weight
    on-device.  It also computes the router-logits output for its
    1/8 slice of tokens in fp32 on the tensor engine.
  - Host scatter-adds the per-expert outputs back to token order.

Layouts on device (per core):
  xt  [D, C]  bf16  gathered tokens, transposed (K on partitions)
  wg  [D, F]  bf16  gate_proj           wu [D, F] bf16  up_proj
  wd  [F, D]  bf16  down_proj
  cw  [C, 1]  f32   combine weight per gathered token (0 = padding)
  xs  [D, TS] f32   this core's token slice, transposed (router input)
  gwt [D, E]  f32   gate weights, transposed (replicated)
outputs:
  yt  [C, D]  f32   weight * expert(token) for the gathered tokens
  lg  [TS, E] f32   router logits slice
"""

import numpy as np
import ml_dtypes

import concourse.bass as bass
import concourse.mybir as mybir
import concourse.tile as tile
from concourse import bacc
from concourse.bass_utils import run_bass_kernel_spmd

B, S, D, F, E = 2, 2048, 1024, 2048, 8
T = B * S
TS = T // E          # tokens per core for the router-logits output
N_CORES = 8
EPS_TIEBREAK = 1e-6
P = 128              # partitions
NBLK = 512           # matmul moving-dim block (one PSUM bank of fp32)

BF16 = mybir.dt.bfloat16
F32 = mybir.dt.float32

TRACE = False        # test.py flips this to capture an NTFF profile
LAST_RESULTS = None  # test.py reads exec_time_ns from here

_compiled = {}       # C -> (nc, names)


def _blocks(total, blk):
    out = []
    o = 0
    while o < total:
        b = min(blk, total - o)
        out.append((o, b))
        o += b
    return out


def build_bass(C):
    KD = D // P    # 8  contraction chunks over D
    KF = F // P    # 16 contraction chunks over F

    nc = bacc.Bacc("TRN2", target_bir_lowering=False, debug=False,
                   num_devices=N_CORES)

    xt = nc.dram_tensor("xt", [D, C], BF16, kind="ExternalInput").ap()
    wg = nc.dram_tensor("wg", [D, F], BF16, kind="ExternalInput").ap()
    wu = nc.dram_tensor("wu", [D, F], BF16, kind="ExternalInput").ap()
    wd = nc.dram_tensor("wd", [F, D], BF16, kind="ExternalInput").ap()
    cw = nc.dram_tensor("cw", [C, 1], F32, kind="ExternalInput").ap()
    xs = nc.dram_tensor("xs", [D, TS], F32, kind="ExternalInput").ap()
    gwt = nc.dram_tensor("gwt", [D, E], F32, kind="ExternalInput").ap()

    yt = nc.dram_tensor("yt", [C, D], F32, kind="ExternalOutput").ap()
    lg = nc.dram_tensor("lg", [TS, E], F32, kind="ExternalOutput").ap()

    cblocks = _blocks(C, NBLK)

    with tile.TileContext(nc) as tc:
        with (
            tc.tile_pool(name="persist", bufs=1) as pp,
            tc.tile_pool(name="work", bufs=3) as wp,
            tc.tile_pool(name="psum", bufs=2, space="PSUM") as pqueue,
            tc.tile_pool(name="psum_lg", bufs=2, space="PSUM") as plg,
        ):
            # ---- persistent loads -------------------------------------
            xt_sb = []
            wg_sb = []
            wu_sb = []
            for dk in range(KD):
                t = pp.tile([P, C], BF16, tag=f"xt{dk}")
                nc.sync.dma_start(out=t, in_=xt[dk * P:(dk + 1) * P, :])
                xt_sb.append(t)
            for dk in range(KD):
                t = pp.tile([P, F], BF16, tag=f"wg{dk}")
                nc.sync.dma_start(out=t, in_=wg[dk * P:(dk + 1) * P, :])
                wg_sb.append(t)
            for dk in range(KD):
                t = pp.tile([P, F], BF16, tag=f"wu{dk}")
                nc.sync.dma_start(out=t, in_=wu[dk * P:(dk + 1) * P, :])
                wu_sb.append(t)
            xs_sb = []
            for dk in range(KD):
                t = pp.tile([P, TS], F32, tag=f"xs{dk}")
                nc.sync.dma_start(out=t, in_=xs[dk * P:(dk + 1) * P, :])
                xs_sb.append(t)
            gwt_sb = []
            for dk in range(KD):
                t = pp.tile([P, E], F32, tag=f"gwt{dk}")
                nc.sync.dma_start(out=t, in_=gwt[dk * P:(dk + 1) * P, :])
                gwt_sb.append(t)
            wd_sb = []
            for fk in range(KF):
                t = pp.tile([P, D], BF16, tag=f"wd{fk}")
                nc.sync.dma_start(out=t, in_=wd[fk * P:(fk + 1) * P, :])
                wd_sb.append(t)
            cw_sb = []
            for ct in range(C // P):
                t = pp.tile([P, 1], F32, tag=f"cw{ct}")
                nc.sync.dma_start(out=t, in_=cw[ct * P:(ct + 1) * P, :])
                cw_sb.append(t)

            # ---- phase B: hT = silu(Wg.T x) * (Wu.T x), [F, C] bf16 ---
            h_sb = [pp.tile([P, C], BF16, tag=f"h{fk}", name=f"h{fk}")
                    for fk in range(KF)]
            for fk in range(KF):
                for (c0, cb) in cblocks:
                    p1 = pqueue.tile([P, NBLK], F32, tag="p1")
                    for dk in range(KD):
                        nc.tensor.matmul(
                            p1[:, :cb],
                            lhsT=wg_sb[dk][:, fk * P:(fk + 1) * P],
                            rhs=xt_sb[dk][:, c0:c0 + cb],
                            start=(dk == 0), stop=(dk == KD - 1),
                        )
                    s1 = wp.tile([P, NBLK], BF16, tag="s1")
                    nc.scalar.activation(s1[:, :cb], p1[:, :cb],
                                         mybir.ActivationFunctionType.Silu)
                    p3 = pqueue.tile([P, NBLK], F32, tag="p3")
                    for dk in range(KD):
                        nc.tensor.matmul(
                            p3[:, :cb],
                            lhsT=wu_sb[dk][:, fk * P:(fk + 1) * P],
                            rhs=xt_sb[dk][:, c0:c0 + cb],
                            start=(dk == 0), stop=(dk == KD - 1),
                        )
                    nc.vector.tensor_mul(h_sb[fk][:, c0:c0 + cb],
                                         s1[:, :cb], p3[:, :cb])

            # ---- phase A: router logits (fp32), [TS, E] ---------------
            for tt in range(TS // P):
                pl = plg.tile([P, E], F32, tag="pl")
                for dk in range(KD):
                    nc.tensor.matmul(
                        pl,
                        lhsT=xs_sb[dk][:, tt * P:(tt + 1) * P],
                        rhs=gwt_sb[dk],
                        start=(dk == 0), stop=(dk == KD - 1),
                    )
                lgs = wp.tile([P, E], F32, tag="lgs")
                nc.scalar.activation(lgs, pl, mybir.ActivationFunctionType.Copy)
                nc.sync.dma_start(out=lg[tt * P:(tt + 1) * P, :], in_=lgs)

            # ---- phase C: yt = cw * (hT.T @ Wd), [C, D] f32 -----------
            for ct in range(C // P):
                for (d0, db) in _blocks(D, NBLK):
                    po = pqueue.tile([P, NBLK], F32, tag="po")
                    for fk in range(KF):
                        nc.tensor.matmul(
                            po[:, :db],
                            lhsT=h_sb[fk][:, ct * P:(ct + 1) * P],
                            rhs=wd_sb[fk][:, d0:d0 + db],
                            start=(fk == 0), stop=(fk == KF - 1),
                        )
                    yts = wp.tile([P, NBLK], F32, tag="yts")
                    nc.scalar.activation(yts[:, :db], po[:, :db],
                                         mybir.ActivationFunctionType.Copy,
                                         scale=cw_sb[ct])
                    nc.sync.dma_start(
                        out=yt[ct * P:(ct + 1) * P, d0:d0 + db],
                        in_=yts[:, :db])

    nc.compile()
    return nc


def _get_compiled(C):
    if C not in _compiled:
        _compiled[C] = build_bass(C)
    return _compiled[C]


def kernel(hidden_states, gate_w, Wg, Wu, Wd, top_k=2, step_num=0, **_):
    global LAST_RESULTS
    assert int(top_k) == 2
    x = np.asarray(hidden_states, dtype=np.float32).reshape(T, D)
    gate_w = np.asarray(gate_w, dtype=np.float32)
    Wg = np.asarray(Wg, dtype=np.float32)
    Wu = np.asarray(Wu, dtype=np.float32)
    Wd = np.asarray(Wd, dtype=np.float32)

    # ---- host routing (sharding decision) ----------------------------
    logits = x @ gate_w.T                                    # [T, E]
    comp = -logits + np.arange(E, dtype=np.float32) * EPS_TIEBREAK
    sel = np.argsort(comp, axis=-1, kind="stable")[:, :2]    # [T, 2]
    sl = np.take_along_axis(logits, sel, axis=-1)
    m = sl.max(axis=-1, keepdims=True)
    ew = np.exp(sl - m)
    rw = (ew / ew.sum(axis=-1, keepdims=True)).astype(np.float32)

    idx = [None] * E
    wts = [None] * E
    for e in range(E):
        rows, cols = np.nonzero(sel == e)
        idx[e] = rows
        wts[e] = rw[rows, cols]
    counts = np.array([len(i) for i in idx])
    C = max(P, int(-(-counts.max() // P)) * P)

    # ---- per-core inputs ---------------------------------------------
    bf = ml_dtypes.bfloat16
    gwt = np.ascontiguousarray(gate_w.T.astype(np.float32))  # [D, E]
    in_maps = []
    for e in range(E):
        n = counts[e]
        xt = np.zeros((D, C), dtype=bf)
        xt[:, :n] = x[idx[e]].T.astype(bf)
        cwv = np.zeros((C, 1), dtype=np.float32)
        cwv[:n, 0] = wts[e]
        in_maps.append({
            "xt": xt,
            "wg": Wg[e].astype(bf),
            "wu": Wu[e].astype(bf),
            "wd": Wd[e].astype(bf),
            "cw": cwv,
            "xs": np.ascontiguousarray(x[e * TS:(e + 1) * TS].T),
            "gwt": gwt,
        })

    nc = _get_compiled(C)
    res = run_bass_kernel_spmd(nc, in_maps, core_ids=list(range(N_CORES)),
                               trace=TRACE)
    LAST_RESULTS = res

    # ---- combine ------------------------------------------------------
    out = np.zeros((T, D), dtype=np.float32)
    for e in range(E):
        n = counts[e]
        out[idx[e]] += res.results[e]["yt"][:n]
    router_logits = np.concatenate(
        [res.results[e]["lg"] for e in range(E)], axis=0)
    return out.reshape(B, S, D), router_logits


# revision 4
# speedup vs baseline: 1.0623x; 1.0623x over previous
"""MoE MLP (top-2 of 8 experts, SwiGLU) on 8 Trainium2 NeuronCores.

Strategy (expert parallelism, per the sharding hint):
  - Host computes router logits once to decide the dispatch (sharding
    decision only), gathers each expert's tokens, and pads to a common
    capacity C.
  - Core e holds expert e's weights (bf16) and runs the 3 matmuls +
    SwiGLU over its gathered tokens, scaling by the combine weight
    on-device.  It also computes the router-logits output for its
    1/8 slice of tokens in fp32 on the tensor engine.
  - Host scatter-adds the per-expert outputs back to token order.

Layouts on device (per core):
  xt  [D, C]  bf16  gathered tokens, transposed (K on partitions)
  wg  [D, F]  bf16  gate_proj           wu [D, F] bf16  up_proj
  wd  [F, D]  bf16  down_proj
  cw  [C, 1]  f32   combine weight per gathered token (0 = padding)
  xs  [D, TS] f32   this core's token slice, transposed (router input)
  gwt [D, E]  f32   gate weights, transposed (replicated)
outputs:
  yt  [C, D]  f32   weight * expert(token) for the gathered tokens
  lg  [TS, E] f32   router logits slice
"""

import numpy as np
import ml_dtypes

import concourse.bass as bass
import concourse.mybir as mybir
import concourse.tile as tile
from concourse import bacc
from concourse.bass_utils import run_bass_kernel_spmd

B, S, D, F, E = 2, 2048, 1024, 2048, 8
T = B * S
TS = T // E          # tokens per core for the router-logits output
N_CORES = 8
EPS_TIEBREAK = 1e-6
P = 128              # partitions
NBLK = 512           # matmul moving-dim block (one PSUM bank of fp32)
WARMUP_MM = 28       # PE warmup matmuls to bridge the input-DMA head

BF16 = mybir.dt.bfloat16
F32 = mybir.dt.float32

TRACE = False        # test.py flips this to capture an NTFF profile
LAST_RESULTS = None  # test.py reads exec_time_ns from here

_compiled = {}       # C -> nc


def _blocks(total, blk):
    out = []
    o = 0
    while o < total:
        b = min(blk, total - o)
        out.append((o, b))
        o += b
    return out


def build_bass(C):
    KD = D // P    # 8  contraction chunks over D
    KF = F // P    # 16 contraction chunks over F
    WBLK = F // 4  # Wg/Wu column-block per DMA (512)

    nc = bacc.Bacc("TRN2", target_bir_lowering=False, debug=False,
                   num_devices=N_CORES)

    xt = nc.dram_tensor("xt", [D, C], BF16, kind="ExternalInput").ap()
    wg = nc.dram_tensor("wg", [D, F], BF16, kind="ExternalInput").ap()
    wu = nc.dram_tensor("wu", [D, F], BF16, kind="ExternalInput").ap()
    wd = nc.dram_tensor("wd", [F, D], BF16, kind="ExternalInput").ap()
    cw = nc.dram_tensor("cw", [C, 1], F32, kind="ExternalInput").ap()
    xs = nc.dram_tensor("xs", [D, TS], F32, kind="ExternalInput").ap()
    gwt = nc.dram_tensor("gwt", [D, E], F32, kind="ExternalInput").ap()

    yt = nc.dram_tensor("yt", [C, D], F32, kind="ExternalOutput").ap()
    lg = nc.dram_tensor("lg", [TS, E], F32, kind="ExternalOutput").ap()

    cblocks = _blocks(C, NBLK)
    ctiles = _blocks(C, P)

    with tile.TileContext(nc) as tc:
        with (
            tc.tile_pool(name="persist", bufs=1) as pp,
            tc.tile_pool(name="work", bufs=3) as wp,
            tc.tile_pool(name="psum", bufs=2, space="PSUM") as pqueue,
            tc.tile_pool(name="psum_lg", bufs=2, space="PSUM") as plg,
        ):
            # ---- loads needed first: router inputs ---------------------
            gwt_sb = []
            for dk in range(KD):
                t = pp.tile([P, E], F32, tag=f"gwt{dk}", name=f"gwt{dk}")
                nc.sync.dma_start(out=t, in_=gwt[dk * P:(dk + 1) * P, :])
                gwt_sb.append(t)
            xs_sb = []
            for dk in range(KD):
                t = pp.tile([P, TS], F32, tag=f"xs{dk}", name=f"xs{dk}")
                nc.sync.dma_start(out=t, in_=xs[dk * P:(dk + 1) * P, :])
                xs_sb.append(t)
            # ---- gathered tokens, then Wg/Wu in column blocks ----------
            xt_sb = []
            for dk in range(KD):
                t = pp.tile([P, C], BF16, tag=f"xt{dk}", name=f"xt{dk}")
                nc.sync.dma_start(out=t, in_=xt[dk * P:(dk + 1) * P, :])
                xt_sb.append(t)
            # wg_sb[blk][dk] / wu_sb[blk][dk]: [P, WBLK] bf16 tiles
            wg_sb = [[None] * KD for _ in range(4)]
            wu_sb = [[None] * KD for _ in range(4)]
            for blk in range(4):
                for dk in range(KD):
                    t = pp.tile([P, WBLK], BF16, tag=f"wg{blk}_{dk}",
                                name=f"wg{blk}_{dk}")
                    nc.sync.dma_start(
                        out=t, in_=wg[dk * P:(dk + 1) * P,
                                      blk * WBLK:(blk + 1) * WBLK])
                    wg_sb[blk][dk] = t
                for dk in range(KD):
                    t = pp.tile([P, WBLK], BF16, tag=f"wu{blk}_{dk}",
                                name=f"wu{blk}_{dk}")
                    nc.sync.dma_start(
                        out=t, in_=wu[dk * P:(dk + 1) * P,
                                      blk * WBLK:(blk + 1) * WBLK])
                    wu_sb[blk][dk] = t
            # ---- tail loads: down-proj weights + combine weights -------
            wd_sb = []
            for fk in range(KF):
                t = pp.tile([P, D], BF16, tag=f"wd{fk}", name=f"wd{fk}")
                nc.sync.dma_start(out=t, in_=wd[fk * P:(fk + 1) * P, :])
                wd_sb.append(t)
            cw_sb = []
            for ci, (t0, tb) in enumerate(ctiles):
                t = pp.tile([P, 1], F32, tag=f"cw{ci}", name=f"cw{ci}")
                nc.sync.dma_start(out=t[:tb, :], in_=cw[t0:t0 + tb, :])
                cw_sb.append(t)

            # ---- PE warmup: keep HAM busy while inputs stream in -------
            wz = pp.tile([P, NBLK], BF16, tag="wz", name="wz")
            nc.vector.memset(wz, 0.0)
            for i in range(WARMUP_MM):
                pw = pqueue.tile([P, NBLK], F32, tag="p1", name=f"pw{i}")
                nc.tensor.matmul(pw, lhsT=wz[:, :P], rhs=wz,
                                 start=True, stop=True)

            # ---- phase A: router logits (fp32), [TS, E] ----------------
            for tt in range(TS // P):
                pl = plg.tile([P, E], F32, tag="pl", name=f"pl{tt}")
                for dk in range(KD):
                    nc.tensor.matmul(
                        pl,
                        lhsT=xs_sb[dk][:, tt * P:(tt + 1) * P],
                        rhs=gwt_sb[dk],
                        start=(dk == 0), stop=(dk == KD - 1),
                    )
                lgs = wp.tile([P, E], F32, tag="lgs", name=f"lgs{tt}")
                nc.scalar.activation(lgs, pl, mybir.ActivationFunctionType.Copy)
                nc.sync.dma_start(out=lg[tt * P:(tt + 1) * P, :], in_=lgs)

            # ---- phase B: hT = silu(Wg.T x) * (Wu.T x), [F, C] bf16 ----
            h_sb = [pp.tile([P, C], BF16, tag=f"h{fk}", name=f"h{fk}")
                    for fk in range(KF)]
            for fk in range(KF):
                blk = fk // 4
                col = (fk % 4) * P
                for (c0, cb) in cblocks:
                    p1 = pqueue.tile([P, NBLK], F32, tag="p1",
                                     name=f"p1_{fk}_{c0}")
                    for dk in range(KD):
                        nc.tensor.matmul(
                            p1[:, :cb],
                            lhsT=wg_sb[blk][dk][:, col:col + P],
                            rhs=xt_sb[dk][:, c0:c0 + cb],
                            start=(dk == 0), stop=(dk == KD - 1),
                        )
                    s1 = wp.tile([P, NBLK], BF16, tag="s1",
                                 name=f"s1_{fk}_{c0}")
                    nc.scalar.activation(s1[:, :cb], p1[:, :cb],
                                         mybir.ActivationFunctionType.Silu)
                    p3 = pqueue.tile([P, NBLK], F32, tag="p3",
                                     name=f"p3_{fk}_{c0}")
                    for dk in range(KD):
                        nc.tensor.matmul(
                            p3[:, :cb],
                            lhsT=wu_sb[blk][dk][:, col:col + P],
                            rhs=xt_sb[dk][:, c0:c0 + cb],
                            start=(dk == 0), stop=(dk == KD - 1),
                        )
                    nc.vector.tensor_mul(h_sb[fk][:, c0:c0 + cb],
                                         s1[:, :cb], p3[:, :cb])

            # ---- phase C: yt = cw * (hT.T @ Wd), [C, D] f32 ------------
            for ci, (t0, tb) in enumerate(ctiles):
                for (d0, db) in _blocks(D, NBLK):
                    po = pqueue.tile([P, NBLK], F32, tag="po",
                                     name=f"po_{ci}_{d0}")
                    for fk in range(KF):
                        nc.tensor.matmul(
                            po[:tb, :db],
                            lhsT=h_sb[fk][:, t0:t0 + tb],
                            rhs=wd_sb[fk][:, d0:d0 + db],
                            start=(fk == 0), stop=(fk == KF - 1),
                        )
                    yts = wp.tile([P, NBLK], F32, tag="yts",
                                  name=f"yts_{ci}_{d0}")
                    nc.scalar.activation(yts[:tb, :db], po[:tb, :db],
                                         mybir.ActivationFunctionType.Copy,
                                         scale=cw_sb[ci][:tb, :])
                    nc.sync.dma_start(
                        out=yt[t0:t0 + tb, d0:d0 + db],
                        in_=yts[:tb, :db])

    nc.compile()
    return nc


def _get_compiled(C):
    if C not in _compiled:
        _compiled[C] = build_bass(C)
    return _compiled[C]


def kernel(hidden_states, gate_w, Wg, Wu, Wd, top_k=2, step_num=0, **_):
    global LAST_RESULTS
    assert int(top_k) == 2
    x = np.asarray(hidden_states, dtype=np.float32).reshape(T, D)
    gate_w = np.asarray(gate_w, dtype=np.float32)
    Wg = np.asarray(Wg, dtype=np.float32)
    Wu = np.asarray(Wu, dtype=np.float32)
    Wd = np.asarray(Wd, dtype=np.float32)

    # ---- host routing (sharding decision) ----------------------------
    logits = x @ gate_w.T                                    # [T, E]
    comp = -logits + np.arange(E, dtype=np.float32) * EPS_TIEBREAK
    sel = np.argsort(comp, axis=-1, kind="stable")[:, :2]    # [T, 2]
    sl = np.take_along_axis(logits, sel, axis=-1)
    m = sl.max(axis=-1, keepdims=True)
    ew = np.exp(sl - m)
    rw = (ew / ew.sum(axis=-1, keepdims=True)).astype(np.float32)

    idx = [None] * E
    wts = [None] * E
    for e in range(E):
        rows, cols = np.nonzero(sel == e)
        idx[e] = rows
        wts[e] = rw[rows, cols]
    counts = np.array([len(i) for i in idx])
    C = max(P, int(-(-counts.max() // 64)) * 64)   # capacity, multiple of 64

    # ---- per-core inputs ---------------------------------------------
    bf = ml_dtypes.bfloat16
    gwt = np.ascontiguousarray(gate_w.T.astype(np.float32))  # [D, E]
    in_maps = []
    for e in range(E):
        n = counts[e]
        xtb = np.zeros((D, C), dtype=bf)
        xtb[:, :n] = x[idx[e]].T.astype(bf)
        cwv = np.zeros((C, 1), dtype=np.float32)
        cwv[:n, 0] = wts[e]
        in_maps.append({
            "xt": xtb,
            "wg": Wg[e].astype(bf),
            "wu": Wu[e].astype(bf),
            "wd": Wd[e].astype(bf),
            "cw": cwv,
            "xs": np.ascontiguousarray(x[e * TS:(e + 1) * TS].T),
            "gwt": gwt,
        })

    nc = _get_compiled(C)
    res = run_bass_kernel_spmd(nc, in_maps, core_ids=list(range(N_CORES)),
                               trace=TRACE)
    LAST_RESULTS = res

    # ---- combine ------------------------------------------------------
    out = np.zeros((T, D), dtype=np.float32)
    for e in range(E):
        n = counts[e]
        out[idx[e]] += res.results[e]["yt"][:n]
    router_logits = np.concatenate(
        [res.results[e]["lg"] for e in range(E)], axis=0)
    return out.reshape(B, S, D), router_logits
